# revision 14
# baseline (speedup 1.0000x reference)
"""Trainium2 Bass kernel for the AugmentedBrownianFollmerSDESTL sampler.

Math (per step i, dt=ts[i+1]-ts[i], gamma=1):
    u   = MLP(x, t_i)  (129->256->256->128, gelu-tanh)
    x  += u*dt + sqrt(gamma*dt)*z_i
    a1 += sqrt(dt) * sum_d(u*z)
    a2 += (dt/2)   * sum_d(u*u)

Device: features on partitions, batch on the free dim; the three layers
chain through the PE (f32r matmuls) with no transposes.  Batch is split
8 ways across cores (512/core) and further into 2 interleaved streams
of 256 so the serial per-step dependency chain of one stream hides
under the other's engine work.  The time conditioning (and b1) enters
as a per-step K=1 matmul into the layer-1 PSUM accumulation, so each
gelu is a single fused ACT op per layer per stream.  The device emits,
per step, the f32r state x_{i+1} and the pre-noise update
xa_i = x_i + dt*u_i; the host recovers u_i = (xa_i - x_i)/dt_i and
computes the two integral channels (a1, a2) with cumsums — they do not
feed the recurrence, so this costs no device time.

Noise is pre-transposed/pre-scaled on the host; the trajectory is
assembled on the host afterwards.
"""

import numpy as np

import concourse.bacc as bacc
import concourse.tile as tile
from concourse import mybir
from concourse.bass_utils import run_bass_kernel_spmd

N_CORES = 8
BATCH = 4096
BL = BATCH // N_CORES  # 512 per core
DIM = 128
HID = 256
NS = 100

F32 = mybir.dt.float32
F32R = mybir.dt.float32r
GELU = mybir.ActivationFunctionType.Gelu_apprx_tanh
MULT = mybir.AluOpType.mult
ADD = mybir.AluOpType.add


def _ts():
    # bit-exact match of jnp.linspace(0.0, 1.0, NS+1, dtype=float32)
    return np.arange(NS + 1, dtype=np.float32) * np.float32(1.0 / NS)


def build(ns=NS, n_streams=2, zero_b2=True, zero_b3=True, fuse_l1=False):
    ts = _ts()
    dts = ts[1:] - ts[:-1]
    S = n_streams
    BS = BL // S

    nc = bacc.Bacc("TRN2", target_bir_lowering=False)
    # zsT: sqrt(dt)-prescaled, transposed noise [ns, DIM, BL]
    zsT = nc.dram_tensor("zsT", [ns, DIM, BL], F32, kind="ExternalInput")
    w1x = nc.dram_tensor("w1x", [DIM, HID], F32R, kind="ExternalInput")
    w2 = nc.dram_tensor("w2", [DIM, 2 * HID], F32R, kind="ExternalInput")
    w3 = nc.dram_tensor("w3", [DIM, 2 * DIM], F32R, kind="ExternalInput")
    # b1eff rows flattened to one partition: [1, ns*HID], col i*HID+m
    b1e = nc.dram_tensor("b1e", [1, ns * HID], F32R, kind="ExternalInput")
    # b1eff as per-partition bias columns: [DIM, 2*ns], col 2i+j
    b1c = nc.dram_tensor("b1c", [DIM, 2 * ns], F32, kind="ExternalInput")
    b23 = nc.dram_tensor("b23", [DIM, 3], F32, kind="ExternalInput")
    # per-step outputs: rows 0..127 = xa (fp32), 128..255 = new state (f32r bits)
    outT = nc.dram_tensor("outT", [ns, 2 * DIM, BL], F32, kind="ExternalOutput")

    zsT_ap, outT_ap = zsT.ap(), outT.ap()

    with tile.TileContext(nc) as tc:
        with (
            tc.tile_pool(name="wpool", bufs=1) as wpool,
            tc.tile_pool(name="zpool", bufs=4) as zpool,
            tc.tile_pool(name="hpool", bufs=2) as hpool,
            tc.tile_pool(name="upool", bufs=2) as upool,
            tc.tile_pool(name="xpool", bufs=3) as xpool,
            tc.tile_pool(name="ppool", bufs=1, space="PSUM") as ppool,
        ):
            w1x_sb = wpool.tile([DIM, HID], F32R)
            nc.sync.dma_start(w1x_sb[:], w1x.ap()[:])
            w2_sb = wpool.tile([DIM, 2 * HID], F32R)
            nc.sync.dma_start(w2_sb[:], w2.ap()[:])
            w3_sb = wpool.tile([DIM, 2 * DIM], F32R)
            nc.sync.dma_start(w3_sb[:], w3.ap()[:])
            if fuse_l1:
                b1e_sb = wpool.tile([1, ns * HID], F32R)
                nc.sync.dma_start(b1e_sb[:], b1e.ap()[:])
            else:
                b1c_sb = wpool.tile([DIM, 2 * ns], F32)
                nc.sync.dma_start(b1c_sb[:], b1c.ap()[:])
            b23_sb = wpool.tile([DIM, 3], F32)
            nc.sync.dma_start(b23_sb[:], b23.ap()[:])
            ones_sb = wpool.tile([1, BS], F32R)
            nc.vector.memset(ones_sb[:].bitcast(F32), 1.0)

            xs = []
            for s in range(S):
                xt = xpool.tile([DIM, BS], F32R, tag=f"x{s}", name=f"x{s}")
                nc.vector.memset(xt[:].bitcast(F32), 0.0)
                xs.append(xt)

            for i in range(ns):
                for s in range(S):
                    c0 = s * BS
                    z = zpool.tile([DIM, BS], F32, tag=f"z{s}", name=f"z{s}")
                    nc.sync.dma_start(z[:], zsT_ap[i, :, c0:c0 + BS])
                    x = xs[s]

                    # layer 1: h1p[j] = b1eff_i[j] + W1x[j]^T x
                    h1p = ppool.tile([DIM, 2 * BS], F32, tag=f"h1p{s}",
                                     name=f"h1p{s}")
                    for j in range(2):
                        sl = slice(j * BS, (j + 1) * BS)
                        if fuse_l1:
                            off = i * HID + j * DIM
                            nc.tensor.matmul(
                                h1p[:, sl], lhsT=b1e_sb[0:1, off:off + DIM],
                                rhs=ones_sb[:], start=True, stop=False)
                            nc.tensor.matmul(
                                h1p[:, sl],
                                lhsT=w1x_sb[:, j * DIM:(j + 1) * DIM],
                                rhs=x[:], start=False, stop=True)
                        else:
                            nc.tensor.matmul(
                                h1p[:, sl],
                                lhsT=w1x_sb[:, j * DIM:(j + 1) * DIM],
                                rhs=x[:], start=True, stop=True)
                    h1 = hpool.tile([DIM, 2 * BS], F32R, tag=f"h1{s}",
                                    name=f"h1{s}")
                    if fuse_l1:
                        nc.scalar.activation(h1[:], h1p[:], GELU)
                    else:
                        for j in range(2):
                            sl = slice(j * BS, (j + 1) * BS)
                            nc.scalar.activation(
                                h1[:, sl], h1p[:, sl], GELU,
                                bias=b1c_sb[:, 2 * i + j:2 * i + j + 1])

                    # layer 2
                    h2p = ppool.tile([DIM, 2 * BS], F32, tag=f"h2p{s}",
                                     name=f"h2p{s}")
                    for j in range(2):
                        sl = slice(j * BS, (j + 1) * BS)
                        for k in range(2):
                            nc.tensor.matmul(
                                h2p[:, sl],
                                lhsT=w2_sb[:, k * HID + j * DIM:
                                           k * HID + (j + 1) * DIM],
                                rhs=h1[:, k * BS:(k + 1) * BS],
                                start=(k == 0), stop=(k == 1))
                    h2 = hpool.tile([DIM, 2 * BS], F32R, tag=f"h2{s}",
                                    name=f"h2{s}")
                    if zero_b2:
                        nc.scalar.activation(h2[:], h2p[:], GELU)
                    else:
                        for j in range(2):
                            sl = slice(j * BS, (j + 1) * BS)
                            nc.scalar.activation(h2[:, sl], h2p[:, sl], GELU,
                                                 bias=b23_sb[:, j:j + 1])

                    # layer 3: up = W3^T h2 (+ b3 handled below)
                    up = ppool.tile([DIM, BS], F32, tag=f"up{s}", name=f"up{s}")
                    for k in range(2):
                        nc.tensor.matmul(
                            up[:], lhsT=w3_sb[:, k * DIM:(k + 1) * DIM],
                            rhs=h2[:, k * BS:(k + 1) * BS],
                            start=(k == 0), stop=(k == 1))

                    # x update: xa = dt*u + x ; x' = zs + xa
                    xa = xpool.tile([DIM, BS], F32, tag=f"xa{s}", name=f"xa{s}")
                    if zero_b3:
                        nc.vector.scalar_tensor_tensor(
                            xa[:], up[:], float(dts[i]), x[:].bitcast(F32),
                            op0=MULT, op1=ADD)
                    else:
                        u = upool.tile([DIM, BS], F32, tag=f"u{s}", name=f"u{s}")
                        nc.vector.tensor_scalar(u[:], up[:], b23_sb[:, 2:3],
                                                None, op0=ADD)
                        nc.vector.scalar_tensor_tensor(
                            xa[:], u[:], float(dts[i]), x[:].bitcast(F32),
                            op0=MULT, op1=ADD)
                    x = xpool.tile([DIM, BS], F32R, tag=f"x{s}", name=f"x{s}")
                    nc.vector.tensor_add(x[:], z[:], xa[:])
                    xs[s] = x

                    nc.sync.dma_start(outT_ap[i, 0:DIM, c0:c0 + BS], xa[:])
                    nc.sync.dma_start(outT_ap[i, DIM:2 * DIM, c0:c0 + BS],
                                      x[:].bitcast(F32))

    nc.compile()
    return nc


_nc_cache = {}


def _get_nc(key=(NS, 2, True, True, False)):
    if key not in _nc_cache:
        _nc_cache[key] = build(*key)
    return _nc_cache[key]


def _host_inputs(noise, W1, b1, W2, b2, W3, b3, ns=NS):
    ts = _ts()
    dts = ts[1:ns + 1] - ts[:ns]
    sqdts = np.sqrt(dts)
    noise = np.asarray(noise, dtype=np.float32)
    W1 = np.asarray(W1, dtype=np.float32)
    b1 = np.asarray(b1, dtype=np.float32)
    W2 = np.asarray(W2, dtype=np.float32)
    W3 = np.asarray(W3, dtype=np.float32)
    b3 = np.asarray(b3, dtype=np.float32)
    b2 = np.asarray(b2, dtype=np.float32)

    w1x = np.ascontiguousarray(W1[:DIM, :])
    w2 = np.concatenate([W2[:DIM, :], W2[DIM:, :]], axis=1)
    w3 = np.concatenate([W3[:DIM, :], W3[DIM:, :]], axis=1)
    # b1eff[i] = b1 + ts[i] * W1[128, :]
    b1eff = b1[None, :] + ts[:ns, None] * W1[DIM, :][None, :]
    b1e = np.ascontiguousarray(b1eff.reshape(1, ns * HID))
    b1c = np.ascontiguousarray(
        b1eff.reshape(ns, 2, DIM).transpose(2, 0, 1).reshape(DIM, 2 * ns))
    b23 = np.stack([b2[:DIM], b2[DIM:], b3], axis=1)

    # noise [ns, BATCH, DIM] -> sqrt(dt)-scaled, per-core [ns, DIM, BL]
    zs = noise[:ns] * sqdts[:, None, None]
    zsT = np.ascontiguousarray(
        zs.reshape(ns, N_CORES, BL, DIM).transpose(1, 0, 3, 2))

    in_maps = []
    for c in range(N_CORES):
        in_maps.append({
            "zsT": zsT[c],
            "w1x": w1x, "w2": w2, "w3": w3, "b1e": b1e, "b1c": b1c,
            "b23": b23,
        })
    return in_maps, zs


def _assemble(results, zs, ns=NS):
    ts = _ts()
    dts = ts[1:ns + 1] - ts[:ns]
    traj = np.zeros((ns + 1, BATCH, DIM + 2), dtype=np.float32)
    # gather xa and state
    xa = np.empty((ns, BATCH, DIM), dtype=np.float32)
    xs = np.empty((ns, BATCH, DIM), dtype=np.float32)
    for c in range(N_CORES):
        o = results[c]["outT"]  # [ns, 2*DIM, BL]
        sl = slice(c * BL, (c + 1) * BL)
        xa[:, sl, :] = o[:, :DIM, :].transpose(0, 2, 1)
        xs[:, sl, :] = o[:, DIM:, :].transpose(0, 2, 1)
    traj[1:, :, :DIM] = xs
    # recover u_i = (xa_i - x_i)/dt_i  (x_0 = 0)
    xprev = np.concatenate([np.zeros((1, BATCH, DIM), np.float32), xs[:-1]], 0)
    u = (xa - xprev) / dts[:, None, None]
    # a1 increments: sum_d u*dW = sum_d u*zs  (gamma=1)
    inc1 = np.einsum("ibd,ibd->ib", u, zs, dtype=np.float32,
                     casting="same_kind")
    # a2 increments: (sum_d u^2)/2 * dt
    inc2 = (np.einsum("ibd,ibd->ib", u, u, dtype=np.float32,
                      casting="same_kind") / np.float32(2.0)) * dts[:, None]
    traj[1:, :, DIM] = np.cumsum(inc1, axis=0, dtype=np.float32)
    traj[1:, :, DIM + 1] = np.cumsum(inc2, axis=0, dtype=np.float32)
    return traj, ts[:ns + 1]


def kernel(noise, W1, b1, W2, b2, W3, b3):
    zero_b2 = bool(np.all(np.asarray(b2) == 0))
    zero_b3 = bool(np.all(np.asarray(b3) == 0))
    nc = _get_nc((NS, 2, zero_b2, zero_b3, False))
    in_maps, zs = _host_inputs(noise, W1, b1, W2, b2, W3, b3)
    res = run_bass_kernel_spmd(nc, in_maps, core_ids=list(range(N_CORES)))
    return _assemble(res.results, zs)


# revision 15
# speedup vs baseline: 2.5703x; 2.5703x over previous
"""Trainium2 Bass kernel for the AugmentedBrownianFollmerSDESTL sampler.

Math (per step i, dt=ts[i+1]-ts[i], gamma=1):
    u   = MLP(x, t_i)  (129->256->256->128, gelu-tanh)
    x  += u*dt + sqrt(gamma*dt)*z_i
    a1 += sqrt(dt) * sum_d(u*z)
    a2 += (dt/2)   * sum_d(u*u)

Device: features on partitions, batch on the free dim; the three layers
chain through the PE (f32r matmuls) with no transposes.  Batch is split
8 ways across cores (512/core) and further into 2 interleaved streams
of 256 so the serial per-step dependency chain of one stream hides
under the other's engine work.  The time conditioning (and b1) enters
as a per-step K=1 matmul into the layer-1 PSUM accumulation, so each
gelu is a single fused ACT op per layer per stream.  The device emits,
per step, the f32r state x_{i+1} and the pre-noise update
xa_i = x_i + dt*u_i; the host recovers u_i = (xa_i - x_i)/dt_i and
computes the two integral channels (a1, a2) with cumsums — they do not
feed the recurrence, so this costs no device time.

Noise is pre-transposed/pre-scaled on the host; the trajectory is
assembled on the host afterwards.
"""

import numpy as np

import concourse.bacc as bacc
import concourse.tile as tile
from concourse import mybir
from concourse.bass_utils import run_bass_kernel_spmd

N_CORES = 8
BATCH = 4096
BL = BATCH // N_CORES  # 512 per core
DIM = 128
HID = 256
NS = 100

F32 = mybir.dt.float32
F32R = mybir.dt.float32r
GELU = mybir.ActivationFunctionType.Gelu_apprx_tanh
MULT = mybir.AluOpType.mult
ADD = mybir.AluOpType.add


def _ts():
    # bit-exact match of jnp.linspace(0.0, 1.0, NS+1, dtype=float32)
    return np.arange(NS + 1, dtype=np.float32) * np.float32(1.0 / NS)


def build(ns=NS, n_streams=2, zero_b2=True, zero_b3=True, fuse_l1=True):
    ts = _ts()
    dts = ts[1:] - ts[:-1]
    S = n_streams
    BS = BL // S

    nc = bacc.Bacc("TRN2", target_bir_lowering=False)
    # zsT: sqrt(dt)-prescaled, transposed noise [ns, DIM, BL]
    zsT = nc.dram_tensor("zsT", [ns, DIM, BL], F32, kind="ExternalInput")
    w1x = nc.dram_tensor("w1x", [DIM, HID], F32R, kind="ExternalInput")
    w2 = nc.dram_tensor("w2", [DIM, 2 * HID], F32R, kind="ExternalInput")
    w3 = nc.dram_tensor("w3", [DIM, 2 * DIM], F32R, kind="ExternalInput")
    # b1eff rows flattened to one partition: [1, ns*HID], col i*HID+m
    b1e = nc.dram_tensor("b1e", [1, ns * HID], F32R, kind="ExternalInput")
    # b1eff as per-partition bias columns: [DIM, 2*ns], col 2i+j
    b1c = nc.dram_tensor("b1c", [DIM, 2 * ns], F32, kind="ExternalInput")
    b23 = nc.dram_tensor("b23", [DIM, 3], F32, kind="ExternalInput")
    # per-step outputs: rows 0..127 = xa (fp32), 128..255 = new state (f32r bits)
    outT = nc.dram_tensor("outT", [ns, 2 * DIM, BL], F32, kind="ExternalOutput")

    zsT_ap, outT_ap = zsT.ap(), outT.ap()

    with tile.TileContext(nc) as tc:
        with (
            tc.tile_pool(name="wpool", bufs=1) as wpool,
            tc.tile_pool(name="zpool", bufs=4) as zpool,
            tc.tile_pool(name="hpool", bufs=2) as hpool,
            tc.tile_pool(name="upool", bufs=2) as upool,
            tc.tile_pool(name="xpool", bufs=3) as xpool,
            tc.tile_pool(name="ppool", bufs=1, space="PSUM") as ppool,
        ):
            w1x_sb = wpool.tile([DIM, HID], F32R)
            nc.sync.dma_start(w1x_sb[:], w1x.ap()[:])
            w2_sb = wpool.tile([DIM, 2 * HID], F32R)
            nc.sync.dma_start(w2_sb[:], w2.ap()[:])
            w3_sb = wpool.tile([DIM, 2 * DIM], F32R)
            nc.sync.dma_start(w3_sb[:], w3.ap()[:])
            if fuse_l1:
                b1e_sb = wpool.tile([1, ns * HID], F32R)
                nc.sync.dma_start(b1e_sb[:], b1e.ap()[:])
            else:
                b1c_sb = wpool.tile([DIM, 2 * ns], F32)
                nc.sync.dma_start(b1c_sb[:], b1c.ap()[:])
            b23_sb = wpool.tile([DIM, 3], F32)
            nc.sync.dma_start(b23_sb[:], b23.ap()[:])
            ones_sb = wpool.tile([1, BS], F32R)
            nc.vector.memset(ones_sb[:].bitcast(F32), 1.0)

            xs = []
            for s in range(S):
                xt = xpool.tile([DIM, BS], F32R, tag=f"x{s}", name=f"x{s}")
                nc.vector.memset(xt[:].bitcast(F32), 0.0)
                xs.append(xt)

            for i in range(ns):
                for s in range(S):
                    c0 = s * BS
                    z = zpool.tile([DIM, BS], F32, tag=f"z{s}", name=f"z{s}")
                    nc.sync.dma_start(z[:], zsT_ap[i, :, c0:c0 + BS])
                    x = xs[s]

                    # layer 1: h1p[j] = b1eff_i[j] + W1x[j]^T x
                    h1p = ppool.tile([DIM, 2 * BS], F32, tag=f"h1p{s}",
                                     name=f"h1p{s}")
                    for j in range(2):
                        sl = slice(j * BS, (j + 1) * BS)
                        if fuse_l1:
                            off = i * HID + j * DIM
                            nc.tensor.matmul(
                                h1p[:, sl], lhsT=b1e_sb[0:1, off:off + DIM],
                                rhs=ones_sb[:], start=True, stop=False)
                            nc.tensor.matmul(
                                h1p[:, sl],
                                lhsT=w1x_sb[:, j * DIM:(j + 1) * DIM],
                                rhs=x[:], start=False, stop=True)
                        else:
                            nc.tensor.matmul(
                                h1p[:, sl],
                                lhsT=w1x_sb[:, j * DIM:(j + 1) * DIM],
                                rhs=x[:], start=True, stop=True)
                    h1 = hpool.tile([DIM, 2 * BS], F32R, tag=f"h1{s}",
                                    name=f"h1{s}")
                    if fuse_l1:
                        nc.scalar.activation(h1[:], h1p[:], GELU)
                    else:
                        for j in range(2):
                            sl = slice(j * BS, (j + 1) * BS)
                            nc.scalar.activation(
                                h1[:, sl], h1p[:, sl], GELU,
                                bias=b1c_sb[:, 2 * i + j:2 * i + j + 1])

                    # layer 2
                    h2p = ppool.tile([DIM, 2 * BS], F32, tag=f"h2p{s}",
                                     name=f"h2p{s}")
                    for j in range(2):
                        sl = slice(j * BS, (j + 1) * BS)
                        for k in range(2):
                            nc.tensor.matmul(
                                h2p[:, sl],
                                lhsT=w2_sb[:, k * HID + j * DIM:
                                           k * HID + (j + 1) * DIM],
                                rhs=h1[:, k * BS:(k + 1) * BS],
                                start=(k == 0), stop=(k == 1))
                    h2 = hpool.tile([DIM, 2 * BS], F32R, tag=f"h2{s}",
                                    name=f"h2{s}")
                    if zero_b2:
                        nc.scalar.activation(h2[:], h2p[:], GELU)
                    else:
                        for j in range(2):
                            sl = slice(j * BS, (j + 1) * BS)
                            nc.scalar.activation(h2[:, sl], h2p[:, sl], GELU,
                                                 bias=b23_sb[:, j:j + 1])

                    # layer 3: up = W3^T h2 (+ b3 handled below)
                    up = ppool.tile([DIM, BS], F32, tag=f"up{s}", name=f"up{s}")
                    for k in range(2):
                        nc.tensor.matmul(
                            up[:], lhsT=w3_sb[:, k * DIM:(k + 1) * DIM],
                            rhs=h2[:, k * BS:(k + 1) * BS],
                            start=(k == 0), stop=(k == 1))

                    # x update: xa = dt*u + x ; x' = zs + xa
                    xa = xpool.tile([DIM, BS], F32, tag=f"xa{s}", name=f"xa{s}")
                    if zero_b3:
                        nc.vector.scalar_tensor_tensor(
                            xa[:], up[:], float(dts[i]), x[:].bitcast(F32),
                            op0=MULT, op1=ADD)
                    else:
                        u = upool.tile([DIM, BS], F32, tag=f"u{s}", name=f"u{s}")
                        nc.vector.tensor_scalar(u[:], up[:], b23_sb[:, 2:3],
                                                None, op0=ADD)
                        nc.vector.scalar_tensor_tensor(
                            xa[:], u[:], float(dts[i]), x[:].bitcast(F32),
                            op0=MULT, op1=ADD)
                    x = xpool.tile([DIM, BS], F32R, tag=f"x{s}", name=f"x{s}")
                    nc.vector.tensor_add(x[:], z[:], xa[:])
                    xs[s] = x

                    nc.sync.dma_start(outT_ap[i, 0:DIM, c0:c0 + BS], xa[:])
                    nc.sync.dma_start(outT_ap[i, DIM:2 * DIM, c0:c0 + BS],
                                      x[:].bitcast(F32))

    nc.compile()
    return nc


_nc_cache = {}


def _get_nc(key=(NS, 2, True, True, True)):
    if key not in _nc_cache:
        _nc_cache[key] = build(*key)
    return _nc_cache[key]


def _host_inputs(noise, W1, b1, W2, b2, W3, b3, ns=NS):
    ts = _ts()
    dts = ts[1:ns + 1] - ts[:ns]
    sqdts = np.sqrt(dts)
    noise = np.asarray(noise, dtype=np.float32)
    W1 = np.asarray(W1, dtype=np.float32)
    b1 = np.asarray(b1, dtype=np.float32)
    W2 = np.asarray(W2, dtype=np.float32)
    W3 = np.asarray(W3, dtype=np.float32)
    b3 = np.asarray(b3, dtype=np.float32)
    b2 = np.asarray(b2, dtype=np.float32)

    w1x = np.ascontiguousarray(W1[:DIM, :])
    w2 = np.concatenate([W2[:DIM, :], W2[DIM:, :]], axis=1)
    w3 = np.concatenate([W3[:DIM, :], W3[DIM:, :]], axis=1)
    # b1eff[i] = b1 + ts[i] * W1[128, :]
    b1eff = b1[None, :] + ts[:ns, None] * W1[DIM, :][None, :]
    b1e = np.ascontiguousarray(b1eff.reshape(1, ns * HID))
    b1c = np.ascontiguousarray(
        b1eff.reshape(ns, 2, DIM).transpose(2, 0, 1).reshape(DIM, 2 * ns))
    b23 = np.stack([b2[:DIM], b2[DIM:], b3], axis=1)

    # noise [ns, BATCH, DIM] -> sqrt(dt)-scaled, per-core [ns, DIM, BL]
    zs = noise[:ns] * sqdts[:, None, None]
    zsT = np.ascontiguousarray(
        zs.reshape(ns, N_CORES, BL, DIM).transpose(1, 0, 3, 2))

    in_maps = []
    for c in range(N_CORES):
        in_maps.append({
            "zsT": zsT[c],
            "w1x": w1x, "w2": w2, "w3": w3, "b1e": b1e, "b1c": b1c,
            "b23": b23,
        })
    return in_maps, zs


def _assemble(results, zs, ns=NS):
    ts = _ts()
    dts = ts[1:ns + 1] - ts[:ns]
    traj = np.zeros((ns + 1, BATCH, DIM + 2), dtype=np.float32)
    # gather xa and state
    xa = np.empty((ns, BATCH, DIM), dtype=np.float32)
    xs = np.empty((ns, BATCH, DIM), dtype=np.float32)
    for c in range(N_CORES):
        o = results[c]["outT"]  # [ns, 2*DIM, BL]
        sl = slice(c * BL, (c + 1) * BL)
        xa[:, sl, :] = o[:, :DIM, :].transpose(0, 2, 1)
        xs[:, sl, :] = o[:, DIM:, :].transpose(0, 2, 1)
    traj[1:, :, :DIM] = xs
    # recover u_i = (xa_i - x_i)/dt_i  (x_0 = 0)
    xprev = np.concatenate([np.zeros((1, BATCH, DIM), np.float32), xs[:-1]], 0)
    u = (xa - xprev) / dts[:, None, None]
    # a1 increments: sum_d u*dW = sum_d u*zs  (gamma=1)
    inc1 = np.einsum("ibd,ibd->ib", u, zs, dtype=np.float32,
                     casting="same_kind")
    # a2 increments: (sum_d u^2)/2 * dt
    inc2 = (np.einsum("ibd,ibd->ib", u, u, dtype=np.float32,
                      casting="same_kind") / np.float32(2.0)) * dts[:, None]
    traj[1:, :, DIM] = np.cumsum(inc1, axis=0, dtype=np.float32)
    traj[1:, :, DIM + 1] = np.cumsum(inc2, axis=0, dtype=np.float32)
    return traj, ts[:ns + 1]


def kernel(noise, W1, b1, W2, b2, W3, b3):
    zero_b2 = bool(np.all(np.asarray(b2) == 0))
    zero_b3 = bool(np.all(np.asarray(b3) == 0))
    nc = _get_nc((NS, 2, zero_b2, zero_b3, True))
    in_maps, zs = _host_inputs(noise, W1, b1, W2, b2, W3, b3)
    res = run_bass_kernel_spmd(nc, in_maps, core_ids=list(range(N_CORES)))
    return _assemble(res.results, zs)
